# revision 28
# baseline (speedup 1.0000x reference)
"""Multi-head attention (B=4, S=2048, D=1024, H=16) on 8 NeuronCores.

Sharding: core c -> (batch b = c//2, head-group g = c%2 of 8 heads).
Per-core: column-parallel fused qkv projection for its 8 heads,
flash-style attention (scores kept transposed: k on partitions so
softmax denominators come from a fused ones-column in the PV matmul),
row-parallel out-projection. The two partial outputs per batch are
summed on the host along with b_out.

Precision: qkv projection accumulates in f32 PSUM; q/k (and v, probs,
ctx, W_out) are stored bf16 (the 2e-2 harness budget has plenty of
room). The 0/1 mask is applied as a bf16 multiply after exp (DVE 2x
mode needs all-2-byte operands).

Phase B uses two manual PSUM rings (scores: 2 slots x [128,1024];
ctx: one slot per head of the pair) so the scores/exp/mask chain runs
a full k-chunk ahead of the PV matmuls; Tile's region-level WAR/RAW
tracking provides the ring synchronization. The mask loads during
phase A's V-projection (qkT kept bf16 to make the SBUF room).
"""
import sys

if "/opt/trn_rl_repo" not in sys.path:
    sys.path.insert(0, "/opt/trn_rl_repo")

import numpy as np

B, S, D, H = 4, 2048, 1024, 16
DH = D // H          # 64
HPC = H // 2         # 8 heads per core
CD = HPC * DH        # 512 local head-dims per core
NCORES = 8

_CACHE = {}


def _split_multiwait(nc):
    """walrus in this container accepts ONE sync wait per instruction;
    hoist extras onto injected same-engine EventSemaphore carriers."""
    import concourse.mybir as mybir

    for fn in nc.m.functions:
        for bb in fn.blocks:
            if not any(
                i.sync_info is not None and i.sync_info.on_wait
                and len(i.sync_info.on_wait) > 1
                for i in bb.instructions
            ):
                continue
            newlist = []
            for inst in bb.instructions:
                si = inst.sync_info
                if si is not None and si.on_wait and len(si.on_wait) > 1:
                    waits = list(si.on_wait)
                    for w in waits[:-1]:
                        ev = mybir.InstEventSemaphore(
                            name=nc.get_next_instruction_name(), ins=[], outs=[])
                        ev.engine = inst.engine
                        ev.sync_info = mybir.SyncInfo(on_wait=[w], on_update=[])
                        newlist.append(ev)
                    inst.sync_info = mybir.SyncInfo(
                        on_wait=[waits[-1]], on_update=list(si.on_update))
                newlist.append(inst)
            try:
                bb.instructions = newlist
            except Exception:
                bb.instructions.clear()
                bb.instructions.extend(newlist)


def build_nc(s=S):
    import concourse.bass as bass
    import concourse.mybir as mybir
    from concourse.tile import TileContext

    F32 = mybir.dt.float32
    F32R = mybir.dt.float32r
    BF16 = mybir.dt.bfloat16
    EXP = mybir.ActivationFunctionType.Exp
    IDENT = mybir.ActivationFunctionType.Identity
    MULT = mybir.AluOpType.mult

    n_sc = s // 128            # s-chunks of 128
    n_st = s // 512            # s-tiles of 512
    n_kc = s // 128            # k chunks (128 each)
    fd_q = min(1024, s)        # q-tile width for attention inner loop
    n_qh = s // fd_q           # q tiles
    n_qn = fd_q // 512         # 512-wide matmuls per q tile
    EW = DH + 2                # per-head vones block (64 V + ones + pad,
                               # 66 keeps bf16 slices 4-byte aligned)
    VW = HPC * EW              # vones row-chunk width
    LA = 2                     # PV lookahead (k-chunks)

    nc = bass.Bass("TRN2", num_devices=NCORES)

    xT = nc.declare_dram_parameter("xT", [D, s], F32R, isOutput=False)
    wqk = nc.declare_dram_parameter("wqk", [D, 2 * CD], F32R, isOutput=False)
    wv = nc.declare_dram_parameter("wv", [D, CD], F32R, isOutput=False)
    bqkt_d = nc.declare_dram_parameter("bqkt", [128, 8], F32R, isOutput=False)
    bv = nc.declare_dram_parameter("bv", [1, CD], F32R, isOutput=False)
    m01 = nc.declare_dram_parameter("m01", [s, s], BF16, isOutput=False)
    wout = nc.declare_dram_parameter("wout", [CD, D], BF16, isOutput=False)
    ones = nc.declare_dram_parameter("ones", [1, 512], F32R, isOutput=False)
    sel_lo = nc.declare_dram_parameter("sel_lo", [1, 128], F32R, isOutput=False)
    sel_hi = nc.declare_dram_parameter("sel_hi", [1, 128], F32R, isOutput=False)
    y = nc.declare_dram_parameter("y", [s, D], F32, isOutput=True)

    with TileContext(nc) as tc:
        with tc.tile_pool(name="persist", bufs=1) as pp:
            qkT = pp.tile([128, 8 * s], BF16, tag="qkT")     # [1024 c, s]
            vones = pp.tile([128, n_sc * VW], BF16, tag="vones")
            sel_lo_t = pp.tile([1, 128], F32R, tag="sel_lo")
            sel_hi_t = pp.tile([1, 128], F32R, tag="sel_hi")
            nc.sync.dma_start(out=sel_lo_t[:], in_=sel_lo[:])
            nc.sync.dma_start(out=sel_hi_t[:], in_=sel_hi[:])

            # ---------------- phase A: qkv projection ----------------
            with tc.tile_pool(name="poolA", bufs=1) as pa, \
                 tc.tile_pool(name="psA", bufs=8, space="PSUM") as psA:
                xt = pa.tile([128, 8 * s], F32R, tag="xt")
                wvt = pa.tile([128, 8 * CD], F32R, tag="wvt")
                ones_row = pa.tile([1, 512], F32R, tag="ones")
                bqkt = pa.tile([128, 8], F32R, tag="bqkt")
                bv_t = pa.tile([1, CD], F32R, tag="bv")

                nc.sync.dma_start(out=ones_row[:], in_=ones[:])
                nc.sync.dma_start(out=bqkt[:], in_=bqkt_d[:])
                nc.sync.dma_start(out=bv_t[:], in_=bv[:])
                xq = [nc.sync, nc.gpsimd]
                for st in range(n_st):
                    for dc in range(8):
                        xq[(st * 8 + dc) % 2].dma_start(
                            out=xt[:, dc * s + st * 512:
                                   dc * s + (st + 1) * 512],
                            in_=xT[dc * 128:(dc + 1) * 128,
                                   st * 512:(st + 1) * 512])
                for dc in range(8):
                    nc.scalar.dma_start(out=wvt[:, dc * CD:(dc + 1) * CD],
                                        in_=wv[dc * 128:(dc + 1) * 128, :])
                # ones columns of vones (the V cols are overwritten below;
                # col DH+1 is alignment padding, also set to 1, never read)
                vones_cols = vones[:].rearrange(
                    "p (ch e) -> p ch e", e=EW)[:, :, DH:DH + 2]
                nc.gpsimd.memset(vones_cols, 1.0)

                # q/k: qkT[c, :] = (W.T x.T), c-tiles of 128; bias fused
                # into the PSUM->SBUF copy (Identity + per-partition bias)
                with tc.tile_pool(name="poolAw", bufs=1) as paw:
                    wqkt = paw.tile([128, 8 * 2 * CD], F32R, tag="wqkt")
                    for dc in range(8):
                        nc.scalar.dma_start(
                            out=wqkt[:, dc * 2 * CD:(dc + 1) * 2 * CD],
                            in_=wqk[dc * 128:(dc + 1) * 128, :])
                    for ct in range(8):
                        pst = [psA.tile([128, 512], F32, tag="pa",
                                        name=f"psqk_{ct}_{st}")
                               for st in range(n_st)]
                        for st in range(n_st):
                            for dc in range(8):
                                nc.tensor.matmul(
                                    pst[st][:],
                                    lhsT=wqkt[:, dc * 2 * CD + ct * 128:
                                              dc * 2 * CD + (ct + 1) * 128],
                                    rhs=xt[:, dc * s + st * 512:
                                           dc * s + (st + 1) * 512],
                                    start=(dc == 0), stop=(dc == 7))
                        for st in range(n_st):
                            nc.scalar.activation(
                                qkT[:, ct * s + st * 512:
                                    ct * s + (st + 1) * 512],
                                pst[st][:], IDENT, bias=bqkt[:, ct:ct + 1])

                # mask loads into the space wqkt freed, overlapping V-proj
                pbm_cm = tc.tile_pool(name="poolBm", bufs=1, side="right")
                pbm = pbm_cm.__enter__()
                m01t = pbm.tile([128, n_kc * s], BF16, tag="m01")
                for kc in range(n_kc):
                    nc.gpsimd.dma_start(out=m01t[:, kc * s:(kc + 1) * s],
                                        in_=m01[kc * 128:(kc + 1) * 128, :])

                # v: natural [s, c] layout, s-chunks of 128, fused ones col
                for scg in range(n_sc // 4):
                    psv = [psA.tile([128, 512], F32, tag="pa",
                                    name=f"psv_{scg}_{i}")
                           for i in range(4)]
                    for dc in range(8):
                        for sci in range(4):
                            sc = scg * 4 + sci
                            nc.tensor.matmul(
                                psv[sci][:],
                                lhsT=xt[:, dc * s + sc * 128:
                                        dc * s + (sc + 1) * 128],
                                rhs=wvt[:, dc * CD:(dc + 1) * CD],
                                start=(dc == 0), stop=False)
                    for sci in range(4):
                        sc = scg * 4 + sci
                        nc.tensor.matmul(
                            psv[sci][:],
                            lhsT=ones_row[0:1, 0:128],
                            rhs=bv_t[0:1, :],
                            start=False, stop=True)
                        dst = vones[:, sc * VW:(sc + 1) * VW].rearrange(
                            "p (h e) -> p h e", e=EW)[:, :, 0:DH]
                        src = psv[sci][:].rearrange("p (h e) -> p h e", e=DH)
                        nc.vector.tensor_copy(dst, src)

            # ---------------- phase B: attention ----------------
            with tc.tile_pool(name="poolB", bufs=1) as pb:
                ctxT = pb.tile([128, 4 * s], BF16, tag="ctxT")   # [512 c, s]
                woutt = pb.tile([128, 4 * D], BF16, tag="wout")
                for ct in range(4):
                    nc.gpsimd.dma_start(out=woutt[:, ct * D:(ct + 1) * D],
                                        in_=wout[ct * 128:(ct + 1) * 128, :])
                battn_cm = tc.tile_pool(name="psB", bufs=1, space="PSUM")
                psB = battn_cm.__enter__()
                pbc_cm = tc.tile_pool(name="poolBc", bufs=2)
                pbc = pbc_cm.__enter__()
                # manual rings everywhere: pool slot reuse is LIFO, which
                # turns a 6-deep pool into an effective depth of 2; explicit
                # round-robin slices + region-level WAR/RAW deps instead.
                # One head at a time (hi outer) -> single 2-bank ctx
                # accumulator, leaving 6 banks = 3 score slots so the exp
                # chain runs 3 deep and the ACT engine never drains.
                NE = 8
                NS = 2
                MMW = min(512, fd_q)
                e_ring = pb.tile([128, NE * fd_q], BF16, tag="ering")
                stg_ring = pb.tile([1, 2 * fd_q], F32R, tag="stgr")
                pss_t = [psB.tile([128, fd_q], F32, tag=f"pss{j}",
                                  name=f"pss{j}")
                         for j in range(NS)]
                ctx_t = [psB.tile([DH + 1, fd_q], F32, tag=f"ctx{j}",
                                  name=f"ctx{j}")
                         for j in range(2)]
                ue = 0

                def make_norm(hp, rs_p, rcp_p, ue0):
                    def emit():
                        # normalize pair hp: ctxT[c, q] *= 1/rowsum
                        with nc.allow_low_precision(
                                reason="f32r recip feeds f32r broadcast mm"):
                            nc.vector.reciprocal(rcp_p[:], rs_p[:])
                        for qh in range(n_qh):
                            r0a = pbc.tile([1, fd_q], F32R, tag="r0",
                                           name=f"r0_{hp}_{qh}")
                            r1a = pbc.tile([1, fd_q], F32R, tag="r1",
                                           name=f"r1_{hp}_{qh}")
                            nc.sync.dma_start(out=r0a[:],
                                              in_=rcp_p[qh:qh + 1, :])
                            nc.sync.dma_start(
                                out=r1a[:],
                                in_=rcp_p[n_qh + qh:n_qh + qh + 1, :])
                            bcp = pss_t[(ue0 + 1 + qh) % NS][:]
                            for n in range(n_qn):
                                nc.tensor.matmul(
                                    bcp[:, n * 512:(n + 1) * 512],
                                    lhsT=sel_lo_t[0:1, :],
                                    rhs=r0a[0:1, n * 512:(n + 1) * 512],
                                    start=True, stop=False)
                                nc.tensor.matmul(
                                    bcp[:, n * 512:(n + 1) * 512],
                                    lhsT=sel_hi_t[0:1, :],
                                    rhs=r1a[0:1, n * 512:(n + 1) * 512],
                                    start=False, stop=True)
                            sl = ctxT[:, hp * s + qh * fd_q:
                                      hp * s + (qh + 1) * fd_q]
                            nc.vector.tensor_tensor(sl, sl, bcp[:], MULT)
                    return emit

                pending_norm = None
                for hp in range(4):
                    h0, h1 = 2 * hp, 2 * hp + 1
                    kt_off = (4 + hp) * s   # K pair c-tile offset in qkT
                    qt_off = hp * s         # Q pair c-tile offset
                    rs_p = pbc.tile([2 * n_qh, fd_q], F32R, tag="rsp",
                                    name=f"rs_{hp}")
                    rcp_p = pbc.tile([2 * n_qh, fd_q], F32R, tag="rcpp",
                                     name=f"rcp_{hp}")
                    for qh in range(n_qh):
                        est_q = []
                        for kcx in range(n_kc + LA):
                            if kcx < n_kc:
                                kc = kcx
                                est = []
                                for hi in range(2):
                                    r0 = 64 * hi
                                    pss = pss_t[hi][:]
                                    for n in range(fd_q // MMW):
                                        nc.tensor.matmul(
                                            pss[:, n * MMW:(n + 1) * MMW],
                                            lhsT=qkT[r0:r0 + 64,
                                                     kt_off + kc * 128:
                                                     kt_off + (kc + 1) * 128],
                                            rhs=qkT[r0:r0 + 64,
                                                    qt_off + qh * fd_q +
                                                    n * MMW:
                                                    qt_off + qh * fd_q +
                                                    (n + 1) * MMW],
                                            start=True, stop=True,
                                            tile_position=(r0, 0))
                                    e = e_ring[:, (ue % NE) * fd_q:
                                               (ue % NE + 1) * fd_q]
                                    ue += 1
                                    nc.scalar.activation(e, pss, EXP)
                                    msl = m01t[:, kc * s + qh * fd_q:
                                               kc * s + (qh + 1) * fd_q]
                                    nc.vector.tensor_tensor(e, e, msl, MULT)
                                    est.append(e)
                                est_q.append((kc, est))
                            if kcx >= LA:
                                kc2, est2 = est_q.pop(0)
                                for hi, h in enumerate((h0, h1)):
                                    for n in range(fd_q // MMW):
                                        nc.tensor.matmul(
                                            ctx_t[hi][:, n * MMW:
                                                      (n + 1) * MMW],
                                            lhsT=vones[:,
                                                       kc2 * VW + h * EW:
                                                       kc2 * VW + h * EW +
                                                       DH + 1],
                                            rhs=est2[hi][:, n * MMW:
                                                         (n + 1) * MMW],
                                            start=(kc2 == 0),
                                            stop=(kc2 == n_kc - 1))
                        # spill unnormalized ctx + rowsums
                        for hi, h in enumerate((h0, h1)):
                            stg = stg_ring[0:1, hi * fd_q:(hi + 1) * fd_q]
                            nc.vector.tensor_copy(stg,
                                                  ctx_t[hi][DH:DH + 1, :])
                            nc.sync.dma_start(
                                out=rs_p[hi * n_qh + qh:
                                         hi * n_qh + qh + 1, :],
                                in_=stg)
                            nc.vector.tensor_copy(
                                ctxT[hi * 64:(hi + 1) * 64,
                                     hp * s + qh * fd_q:
                                     hp * s + (qh + 1) * fd_q],
                                ctx_t[hi][0:DH, :])
                        if qh == 0 and pending_norm is not None:
                            pending_norm()
                            pending_norm = None
                    pending_norm = make_norm(hp, rs_p, rcp_p, ue)
                if pending_norm is not None:
                    pending_norm()

                pbc_cm.__exit__(None, None, None)
                battn_cm.__exit__(None, None, None)
                # ---------------- phase C: out projection ----------------
                with (
                    tc.tile_pool(name="poolC", bufs=1) as pc,
                    tc.tile_pool(name="psC", bufs=1, space="PSUM") as psC,
                ):
                    NC = 4
                    po_t = [psC.tile([128, D], F32, tag=f"po{j}",
                                     name=f"po{j}")
                            for j in range(NC)]
                    ot_ring = pc.tile([128, NC * D], F32, tag="otr")
                    yq = [nc.sync, nc.gpsimd]
                    for qc in range(n_sc):
                        po = po_t[qc % NC]
                        for ct in range(4):
                            for n in range(D // MMW):
                                nc.tensor.matmul(
                                    po[:, n * MMW:(n + 1) * MMW],
                                    lhsT=ctxT[:, ct * s + qc * 128:
                                              ct * s + (qc + 1) * 128
                                              ],
                                    rhs=woutt[:, ct * D + n * MMW:
                                              ct * D + (n + 1) * MMW
                                              ],
                                    start=(ct == 0), stop=(ct == 3))
                        ot = ot_ring[:, (qc % NC) * D:(qc % NC + 1) * D]
                        nc.scalar.copy(out=ot, in_=po[:])
                        yq[qc % 2].dma_start(
                            out=y[qc * 128:(qc + 1) * 128, :],
                            in_=ot)
            pbm_cm.__exit__(None, None, None)

    _split_multiwait(nc)
    return nc


def _get_nc(s=S):
    if s not in _CACHE:
        _CACHE[s] = build_nc(s)
    return _CACHE[s]


def make_in_maps(x, W_qkv, b_qkv, W_out, mask, s=S):
    import ml_dtypes

    BF = ml_dtypes.bfloat16
    x = np.asarray(x, dtype=np.float32)
    W_qkv = np.asarray(W_qkv, dtype=np.float32)
    b_qkv = np.asarray(b_qkv, dtype=np.float32)
    W_out = np.asarray(W_out, dtype=np.float32)
    mask = np.asarray(mask)
    scale = 1.0 / np.sqrt(DH)
    m01 = np.ascontiguousarray((mask[0, 0] != 0).T.astype(BF))
    in_maps = []
    for c in range(NCORES):
        b, g = c // 2, c % 2
        wq = W_qkv[:, g * CD:(g + 1) * CD] * scale
        wk = W_qkv[:, D + g * CD:D + (g + 1) * CD]
        bqk = np.concatenate(
            [b_qkv[g * CD:(g + 1) * CD] * scale,
             b_qkv[D + g * CD:D + (g + 1) * CD]])
        in_maps.append({
            "xT": np.ascontiguousarray(x[b].T),
            "wqk": np.ascontiguousarray(np.concatenate([wq, wk], axis=1)),
            "wv": np.ascontiguousarray(
                W_qkv[:, 2 * D + g * CD:2 * D + (g + 1) * CD]),
            "bqkt": np.ascontiguousarray(bqk.reshape(8, 128).T),
            "bv": np.ascontiguousarray(
                b_qkv[2 * D + g * CD:2 * D + (g + 1) * CD][None, :]),
            "m01": m01,
            "wout": np.ascontiguousarray(
                W_out[g * CD:(g + 1) * CD, :].astype(BF)),
            "ones": np.ones((1, 512), dtype=np.float32),
            "sel_lo": np.concatenate(
                [np.ones(64), np.zeros(64)])[None, :].astype(np.float32),
            "sel_hi": np.concatenate(
                [np.zeros(64), np.ones(64)])[None, :].astype(np.float32),
        })
    return in_maps


def kernel(x, W_qkv, b_qkv, W_out, b_out, mask):
    from concourse.bass_utils import run_bass_kernel_spmd

    nc = _get_nc(S)
    in_maps = make_in_maps(x, W_qkv, b_qkv, W_out, mask, S)
    res = run_bass_kernel_spmd(nc, in_maps, list(range(NCORES)))
    b_out = np.asarray(b_out, dtype=np.float32)
    y = np.empty((B, S, D), dtype=np.float32)
    for b in range(B):
        y[b] = res.results[2 * b]["y"] + res.results[2 * b + 1]["y"] + b_out
    return y


# revision 29
# speedup vs baseline: 1.1930x; 1.1930x over previous
"""Multi-head attention (B=4, S=2048, D=1024, H=16) on 8 NeuronCores.

Sharding: core c -> (batch b = c//2, head-group g = c%2 of 8 heads).
Per-core: column-parallel fused qkv projection for its 8 heads,
flash-style attention (scores kept transposed: k on partitions so
softmax denominators come from a fused ones-column in the PV matmul),
row-parallel out-projection. The two partial outputs per batch are
summed on the host along with b_out.

Precision: qkv projection accumulates in f32 PSUM; q/k (and v, probs,
ctx, W_out) are stored bf16 (the 2e-2 harness budget has plenty of
room). The 0/1 mask is applied as a bf16 multiply after exp (DVE 2x
mode needs all-2-byte operands).

Phase B uses two manual PSUM rings (scores: 2 slots x [128,1024];
ctx: one slot per head of the pair) so the scores/exp/mask chain runs
a full k-chunk ahead of the PV matmuls; Tile's region-level WAR/RAW
tracking provides the ring synchronization. The mask loads during
phase A's V-projection (qkT kept bf16 to make the SBUF room).
"""
import sys

if "/opt/trn_rl_repo" not in sys.path:
    sys.path.insert(0, "/opt/trn_rl_repo")

import numpy as np

B, S, D, H = 4, 2048, 1024, 16
DH = D // H          # 64
HPC = H // 2         # 8 heads per core
CD = HPC * DH        # 512 local head-dims per core
NCORES = 8

_CACHE = {}


def _split_multiwait(nc):
    """walrus in this container accepts ONE sync wait per instruction;
    hoist extras onto injected same-engine EventSemaphore carriers."""
    import concourse.mybir as mybir

    for fn in nc.m.functions:
        for bb in fn.blocks:
            if not any(
                i.sync_info is not None and i.sync_info.on_wait
                and len(i.sync_info.on_wait) > 1
                for i in bb.instructions
            ):
                continue
            newlist = []
            for inst in bb.instructions:
                si = inst.sync_info
                if si is not None and si.on_wait and len(si.on_wait) > 1:
                    waits = list(si.on_wait)
                    for w in waits[:-1]:
                        ev = mybir.InstEventSemaphore(
                            name=nc.get_next_instruction_name(), ins=[], outs=[])
                        ev.engine = inst.engine
                        ev.sync_info = mybir.SyncInfo(on_wait=[w], on_update=[])
                        newlist.append(ev)
                    inst.sync_info = mybir.SyncInfo(
                        on_wait=[waits[-1]], on_update=list(si.on_update))
                newlist.append(inst)
            try:
                bb.instructions = newlist
            except Exception:
                bb.instructions.clear()
                bb.instructions.extend(newlist)


def build_nc(s=S):
    import concourse.bass as bass
    import concourse.mybir as mybir
    from concourse.tile import TileContext

    F32 = mybir.dt.float32
    F32R = mybir.dt.float32r
    BF16 = mybir.dt.bfloat16
    EXP = mybir.ActivationFunctionType.Exp
    IDENT = mybir.ActivationFunctionType.Identity
    MULT = mybir.AluOpType.mult

    n_sc = s // 128            # s-chunks of 128
    n_st = s // 512            # s-tiles of 512
    n_kc = s // 128            # k chunks (128 each)
    fd_q = min(1024, s)        # q-tile width for attention inner loop
    n_qh = s // fd_q           # q tiles
    n_qn = fd_q // 512         # 512-wide matmuls per q tile
    EW = DH + 2                # per-head vones block (64 V + ones + pad,
                               # 66 keeps bf16 slices 4-byte aligned)
    VW = HPC * EW              # vones row-chunk width
    LA = 2                     # PV lookahead (k-chunks)

    nc = bass.Bass("TRN2", num_devices=NCORES)

    xT = nc.declare_dram_parameter("xT", [D, s], F32R, isOutput=False)
    wqk = nc.declare_dram_parameter("wqk", [D, 2 * CD], F32R, isOutput=False)
    wv = nc.declare_dram_parameter("wv", [D, CD], F32R, isOutput=False)
    bqkt_d = nc.declare_dram_parameter("bqkt", [128, 8], F32R, isOutput=False)
    bv = nc.declare_dram_parameter("bv", [1, CD], F32R, isOutput=False)
    m01 = nc.declare_dram_parameter("m01", [s, s], BF16, isOutput=False)
    wout = nc.declare_dram_parameter("wout", [CD, D], BF16, isOutput=False)
    ones = nc.declare_dram_parameter("ones", [1, 512], F32R, isOutput=False)
    sel_lo = nc.declare_dram_parameter("sel_lo", [1, 128], F32R, isOutput=False)
    sel_hi = nc.declare_dram_parameter("sel_hi", [1, 128], F32R, isOutput=False)
    y = nc.declare_dram_parameter("y", [s, D], F32, isOutput=True)

    with TileContext(nc) as tc:
        with tc.tile_pool(name="persist", bufs=1) as pp:
            qkT = pp.tile([128, 8 * s], BF16, tag="qkT")     # [1024 c, s]
            vones = pp.tile([128, n_sc * VW], BF16, tag="vones")
            sel_lo_t = pp.tile([1, 128], F32R, tag="sel_lo")
            sel_hi_t = pp.tile([1, 128], F32R, tag="sel_hi")
            nc.sync.dma_start(out=sel_lo_t[:], in_=sel_lo[:])
            nc.sync.dma_start(out=sel_hi_t[:], in_=sel_hi[:])

            # ---------------- phase A: qkv projection ----------------
            with tc.tile_pool(name="poolA", bufs=1) as pa, \
                 tc.tile_pool(name="psA", bufs=8, space="PSUM") as psA:
                xt = pa.tile([128, 8 * s], F32R, tag="xt")
                wvt = pa.tile([128, 8 * CD], F32R, tag="wvt")
                ones_row = pa.tile([1, 512], F32R, tag="ones")
                bqkt = pa.tile([128, 8], F32R, tag="bqkt")
                bv_t = pa.tile([1, CD], F32R, tag="bv")

                nc.sync.dma_start(out=ones_row[:], in_=ones[:])
                nc.sync.dma_start(out=bqkt[:], in_=bqkt_d[:])
                nc.sync.dma_start(out=bv_t[:], in_=bv[:])
                xq = [nc.sync, nc.gpsimd]
                for st in range(n_st):
                    for dc in range(8):
                        xq[(st * 8 + dc) % 2].dma_start(
                            out=xt[:, dc * s + st * 512:
                                   dc * s + (st + 1) * 512],
                            in_=xT[dc * 128:(dc + 1) * 128,
                                   st * 512:(st + 1) * 512])
                for dc in range(8):
                    nc.scalar.dma_start(out=wvt[:, dc * CD:(dc + 1) * CD],
                                        in_=wv[dc * 128:(dc + 1) * 128, :])
                # ones columns of vones (the V cols are overwritten below;
                # col DH+1 is alignment padding, also set to 1, never read)
                vones_cols = vones[:].rearrange(
                    "p (ch e) -> p ch e", e=EW)[:, :, DH:DH + 2]
                nc.gpsimd.memset(vones_cols, 1.0)

                # q/k: qkT[c, :] = (W.T x.T), c-tiles of 128; bias fused
                # into the PSUM->SBUF copy (Identity + per-partition bias)
                with tc.tile_pool(name="poolAw", bufs=1) as paw:
                    wqkt = paw.tile([128, 8 * 2 * CD], F32R, tag="wqkt")
                    for dc in range(8):
                        nc.scalar.dma_start(
                            out=wqkt[:, dc * 2 * CD:(dc + 1) * 2 * CD],
                            in_=wqk[dc * 128:(dc + 1) * 128, :])
                    for ct in range(8):
                        pst = [psA.tile([128, 512], F32, tag="pa",
                                        name=f"psqk_{ct}_{st}")
                               for st in range(n_st)]
                        for st in range(n_st):
                            for dc in range(8):
                                nc.tensor.matmul(
                                    pst[st][:],
                                    lhsT=wqkt[:, dc * 2 * CD + ct * 128:
                                              dc * 2 * CD + (ct + 1) * 128],
                                    rhs=xt[:, dc * s + st * 512:
                                           dc * s + (st + 1) * 512],
                                    start=(dc == 0), stop=(dc == 7))
                        for st in range(n_st):
                            nc.scalar.activation(
                                qkT[:, ct * s + st * 512:
                                    ct * s + (st + 1) * 512],
                                pst[st][:], IDENT, bias=bqkt[:, ct:ct + 1])

                # mask loads into the space wqkt freed, overlapping V-proj
                pbm_cm = tc.tile_pool(name="poolBm", bufs=1, side="right")
                pbm = pbm_cm.__enter__()
                m01t = pbm.tile([128, n_kc * s], BF16, tag="m01")
                for kc in range(n_kc):
                    nc.gpsimd.dma_start(out=m01t[:, kc * s:(kc + 1) * s],
                                        in_=m01[kc * 128:(kc + 1) * 128, :])

                # v: natural [s, c] layout, s-chunks of 128, fused ones col
                for scg in range(n_sc // 4):
                    psv = [psA.tile([128, 512], F32, tag="pa",
                                    name=f"psv_{scg}_{i}")
                           for i in range(4)]
                    for dc in range(8):
                        for sci in range(4):
                            sc = scg * 4 + sci
                            nc.tensor.matmul(
                                psv[sci][:],
                                lhsT=xt[:, dc * s + sc * 128:
                                        dc * s + (sc + 1) * 128],
                                rhs=wvt[:, dc * CD:(dc + 1) * CD],
                                start=(dc == 0), stop=False)
                    for sci in range(4):
                        sc = scg * 4 + sci
                        nc.tensor.matmul(
                            psv[sci][:],
                            lhsT=ones_row[0:1, 0:128],
                            rhs=bv_t[0:1, :],
                            start=False, stop=True)
                        dst = vones[:, sc * VW:(sc + 1) * VW].rearrange(
                            "p (h e) -> p h e", e=EW)[:, :, 0:DH]
                        src = psv[sci][:].rearrange("p (h e) -> p h e", e=DH)
                        nc.vector.tensor_copy(dst, src)

            # ---------------- phase B: attention ----------------
            with tc.tile_pool(name="poolB", bufs=1) as pb:
                ctxT = pb.tile([128, 4 * s], BF16, tag="ctxT")   # [512 c, s]
                woutt = pb.tile([128, 4 * D], BF16, tag="wout")
                for ct in range(4):
                    nc.gpsimd.dma_start(out=woutt[:, ct * D:(ct + 1) * D],
                                        in_=wout[ct * 128:(ct + 1) * 128, :])
                battn_cm = tc.tile_pool(name="psB", bufs=1, space="PSUM")
                psB = battn_cm.__enter__()
                pbc_cm = tc.tile_pool(name="poolBc", bufs=2)
                pbc = pbc_cm.__enter__()
                # manual rings everywhere: pool slot reuse is LIFO, which
                # turns a 6-deep pool into an effective depth of 2; explicit
                # round-robin slices + region-level WAR/RAW deps instead.
                # One head at a time (hi outer) -> single 2-bank ctx
                # accumulator, leaving 6 banks = 3 score slots so the exp
                # chain runs 3 deep and the ACT engine never drains.
                NE = 8
                NS = 3
                MMW = min(512, fd_q)
                e_ring = pb.tile([128, NE * fd_q], BF16, tag="ering")
                stg_ring = pb.tile([1, 2 * fd_q], F32R, tag="stgr")
                pss_t = [psB.tile([128, fd_q], F32, tag=f"pss{j}",
                                  name=f"pss{j}")
                         for j in range(NS)]
                ctx_ring = psB.tile([DH + 1, fd_q], F32, tag="ctxr")
                ue = 0

                def make_norm(hp, rs_p, rcp_p, ue0):
                    def emit():
                        # normalize pair hp: ctxT[c, q] *= 1/rowsum
                        with nc.allow_low_precision(
                                reason="f32r recip feeds f32r broadcast mm"):
                            nc.vector.reciprocal(rcp_p[:], rs_p[:])
                        for qh in range(n_qh):
                            r0a = pbc.tile([1, fd_q], F32R, tag="r0",
                                           name=f"r0_{hp}_{qh}")
                            r1a = pbc.tile([1, fd_q], F32R, tag="r1",
                                           name=f"r1_{hp}_{qh}")
                            nc.sync.dma_start(out=r0a[:],
                                              in_=rcp_p[qh:qh + 1, :])
                            nc.sync.dma_start(
                                out=r1a[:],
                                in_=rcp_p[n_qh + qh:n_qh + qh + 1, :])
                            bcp = pss_t[(ue0 + 1 + qh) % NS][:]
                            for n in range(n_qn):
                                nc.tensor.matmul(
                                    bcp[:, n * 512:(n + 1) * 512],
                                    lhsT=sel_lo_t[0:1, :],
                                    rhs=r0a[0:1, n * 512:(n + 1) * 512],
                                    start=True, stop=False)
                                nc.tensor.matmul(
                                    bcp[:, n * 512:(n + 1) * 512],
                                    lhsT=sel_hi_t[0:1, :],
                                    rhs=r1a[0:1, n * 512:(n + 1) * 512],
                                    start=False, stop=True)
                            sl = ctxT[:, hp * s + qh * fd_q:
                                      hp * s + (qh + 1) * fd_q]
                            nc.vector.tensor_tensor(sl, sl, bcp[:], MULT)
                    return emit

                pending_norm = None
                for hp in range(4):
                    h0, h1 = 2 * hp, 2 * hp + 1
                    kt_off = (4 + hp) * s   # K pair c-tile offset in qkT
                    qt_off = hp * s         # Q pair c-tile offset
                    rs_p = pbc.tile([2 * n_qh, fd_q], F32R, tag="rsp",
                                    name=f"rs_{hp}")
                    rcp_p = pbc.tile([2 * n_qh, fd_q], F32R, tag="rcpp",
                                     name=f"rcp_{hp}")
                    for qh in range(n_qh):
                        for hi, h in enumerate((h0, h1)):
                            r0 = 64 * hi
                            est_q = []
                            for kcx in range(n_kc + LA):
                                if kcx < n_kc:
                                    kc = kcx
                                    pss = pss_t[ue % NS][:]
                                    for n in range(fd_q // MMW):
                                        nc.tensor.matmul(
                                            pss[:, n * MMW:(n + 1) * MMW],
                                            lhsT=qkT[r0:r0 + 64,
                                                     kt_off + kc * 128:
                                                     kt_off + (kc + 1) * 128],
                                            rhs=qkT[r0:r0 + 64,
                                                    qt_off + qh * fd_q +
                                                    n * MMW:
                                                    qt_off + qh * fd_q +
                                                    (n + 1) * MMW],
                                            start=True, stop=True,
                                            tile_position=(r0, 0))
                                    e = e_ring[:, (ue % NE) * fd_q:
                                               (ue % NE + 1) * fd_q]
                                    ue += 1
                                    nc.scalar.activation(e, pss, EXP)
                                    msl = m01t[:, kc * s + qh * fd_q:
                                               kc * s + (qh + 1) * fd_q]
                                    nc.vector.tensor_tensor(e, e, msl, MULT)
                                    est_q.append((kc, e))
                                if kcx >= LA:
                                    kc2, e2 = est_q.pop(0)
                                    for n in range(fd_q // MMW):
                                        nc.tensor.matmul(
                                            ctx_ring[:, n * MMW:
                                                     (n + 1) * MMW],
                                            lhsT=vones[:,
                                                       kc2 * VW + h * EW:
                                                       kc2 * VW + h * EW +
                                                       DH + 1],
                                            rhs=e2[:, n * MMW:(n + 1) * MMW],
                                            start=(kc2 == 0),
                                            stop=(kc2 == n_kc - 1))
                            # spill unnormalized ctx + rowsums
                            stg = stg_ring[0:1, hi * fd_q:(hi + 1) * fd_q]
                            nc.vector.tensor_copy(stg,
                                                  ctx_ring[DH:DH + 1, :])
                            nc.sync.dma_start(
                                out=rs_p[hi * n_qh + qh:
                                         hi * n_qh + qh + 1, :],
                                in_=stg)
                            nc.vector.tensor_copy(
                                ctxT[hi * 64:(hi + 1) * 64,
                                     hp * s + qh * fd_q:
                                     hp * s + (qh + 1) * fd_q],
                                ctx_ring[0:DH, :])
                        if qh == 0 and pending_norm is not None:
                            pending_norm()
                            pending_norm = None
                    pending_norm = make_norm(hp, rs_p, rcp_p, ue)
                if pending_norm is not None:
                    pending_norm()

                pbc_cm.__exit__(None, None, None)
                battn_cm.__exit__(None, None, None)
                # ---------------- phase C: out projection ----------------
                with (
                    tc.tile_pool(name="poolC", bufs=1) as pc,
                    tc.tile_pool(name="psC", bufs=1, space="PSUM") as psC,
                ):
                    NC = 4
                    po_t = [psC.tile([128, D], F32, tag=f"po{j}",
                                     name=f"po{j}")
                            for j in range(NC)]
                    ot_ring = pc.tile([128, NC * D], F32, tag="otr")
                    yq = [nc.sync, nc.gpsimd]
                    for qc in range(n_sc):
                        po = po_t[qc % NC]
                        for ct in range(4):
                            for n in range(D // MMW):
                                nc.tensor.matmul(
                                    po[:, n * MMW:(n + 1) * MMW],
                                    lhsT=ctxT[:, ct * s + qc * 128:
                                              ct * s + (qc + 1) * 128
                                              ],
                                    rhs=woutt[:, ct * D + n * MMW:
                                              ct * D + (n + 1) * MMW
                                              ],
                                    start=(ct == 0), stop=(ct == 3))
                        ot = ot_ring[:, (qc % NC) * D:(qc % NC + 1) * D]
                        nc.scalar.copy(out=ot, in_=po[:])
                        yq[qc % 2].dma_start(
                            out=y[qc * 128:(qc + 1) * 128, :],
                            in_=ot)
            pbm_cm.__exit__(None, None, None)

    _split_multiwait(nc)
    return nc


def _get_nc(s=S):
    if s not in _CACHE:
        _CACHE[s] = build_nc(s)
    return _CACHE[s]


def make_in_maps(x, W_qkv, b_qkv, W_out, mask, s=S):
    import ml_dtypes

    BF = ml_dtypes.bfloat16
    x = np.asarray(x, dtype=np.float32)
    W_qkv = np.asarray(W_qkv, dtype=np.float32)
    b_qkv = np.asarray(b_qkv, dtype=np.float32)
    W_out = np.asarray(W_out, dtype=np.float32)
    mask = np.asarray(mask)
    scale = 1.0 / np.sqrt(DH)
    m01 = np.ascontiguousarray((mask[0, 0] != 0).T.astype(BF))
    in_maps = []
    for c in range(NCORES):
        b, g = c // 2, c % 2
        wq = W_qkv[:, g * CD:(g + 1) * CD] * scale
        wk = W_qkv[:, D + g * CD:D + (g + 1) * CD]
        bqk = np.concatenate(
            [b_qkv[g * CD:(g + 1) * CD] * scale,
             b_qkv[D + g * CD:D + (g + 1) * CD]])
        in_maps.append({
            "xT": np.ascontiguousarray(x[b].T),
            "wqk": np.ascontiguousarray(np.concatenate([wq, wk], axis=1)),
            "wv": np.ascontiguousarray(
                W_qkv[:, 2 * D + g * CD:2 * D + (g + 1) * CD]),
            "bqkt": np.ascontiguousarray(bqk.reshape(8, 128).T),
            "bv": np.ascontiguousarray(
                b_qkv[2 * D + g * CD:2 * D + (g + 1) * CD][None, :]),
            "m01": m01,
            "wout": np.ascontiguousarray(
                W_out[g * CD:(g + 1) * CD, :].astype(BF)),
            "ones": np.ones((1, 512), dtype=np.float32),
            "sel_lo": np.concatenate(
                [np.ones(64), np.zeros(64)])[None, :].astype(np.float32),
            "sel_hi": np.concatenate(
                [np.zeros(64), np.ones(64)])[None, :].astype(np.float32),
        })
    return in_maps


def kernel(x, W_qkv, b_qkv, W_out, b_out, mask):
    from concourse.bass_utils import run_bass_kernel_spmd

    nc = _get_nc(S)
    in_maps = make_in_maps(x, W_qkv, b_qkv, W_out, mask, S)
    res = run_bass_kernel_spmd(nc, in_maps, list(range(NCORES)))
    b_out = np.asarray(b_out, dtype=np.float32)
    y = np.empty((B, S, D), dtype=np.float32)
    for b in range(B):
        y[b] = res.results[2 * b]["y"] + res.results[2 * b + 1]["y"] + b_out
    return y
